# revision 26
# baseline (speedup 1.0000x reference)
"""Trainium2 Bass kernel for 2-layer RGCN (mean aggregation) on 8 NeuronCores.

Design:
  - dst-sharded: core k owns 6250 destination rows, permuted into 52 tiles of
    128 via a balanced 16-dimensional bin-packing (8 relations x 2 source
    halves) so nearly every (relation, tile, half) group fits ONE chunk of
    128 edge slots. The per-tile chunk schedule is shared by all cores to
    keep the program SPMD-uniform.
  - Layer-1 per-edge messages (x[src] in edge-slot order) are materialized
    on the host as part of input sharding (per the sharding hint) and
    streamed to SBUF with plain sequential DMA. Layer-2 messages depend on
    device-computed h, so they are fetched with per-tile dma_gather pairs:
    the h table is split at row 26624 (= 4 padded core blocks = src<25000,
    the same edge split in both layers) so each half fits dma_gather's
    signed-int16 index range; every chunk is purely lo or purely hi. Pad
    slots gather row 0 and are masked out downstream. Pool-engine gather
    descriptor generation costs ~8.6 ns/row and is layer 2's floor.
  - Per chunk: a host-precomputed mask [128e, 128d] (one-hot with the mean
    normalization 1/cnt folded in, streamed by DMA) and FC TensorE matmuls
    msgs_chunk^T @ mask accumulate the transposed per-(r, tile) segment
    sums [feature, dst] directly in PSUM - no on-device transposes needed.
  - The per-relation transform (agg @ W_r summed over r) uses the PSUM->SBUF
    aggregate copies (alternating Scalar/Vector engines) as the stationary
    operand; root term and bias are extra accumulating matmuls into the
    same PSUM tile.
  - One launch per layer; the host rebuilds the transposed root operand and
    the layer-2 gather table between launches. All matmul operands are bf16
    with fp32 PSUM accumulation.
"""
import numpy as np
import ml_dtypes

N = 50000
E = 800000
R = 8
IN, HID, OUT = 512, 256, 512
NCORES = 8
SHARD = 6250
NT = 52                    # tiles per core
NOVER = 3                  # overflow-capable tiles (cap 256 instead of 128)
PSH = NT * 128             # 6656 padded rows per core
SPL1 = 25000               # layer-1 lo/hi source split
SPL2 = 4 * PSH             # layer-2 split: same edges (src<25000) by design
bf16 = ml_dtypes.bfloat16

_pending_trace = {"l1": None, "l2": None}
_last_traced = None


# ---------------------------------------------------------------------------
# Workarounds for this container's walrus build (single sync-wait per
# instruction) and missing NTFF profile hook under axon.
# ---------------------------------------------------------------------------
def _split_multiwaits(nc):
    import concourse.mybir as mybir
    counter = [0]
    for f in nc.m.functions:
        for bb in f.blocks:
            out = []
            changed = False
            for inst in bb.instructions:
                si = inst.sync_info
                waits = list(si.on_wait) if si is not None else []
                if len(waits) > 1:
                    changed = True
                    for w in waits[:-1]:
                        counter[0] += 1
                        nop = mybir.InstNoOp(
                            name=f"I-wsplit-{inst.name}-{counter[0]}",
                            ins=[], outs=[])
                        nop.engine = inst.engine
                        nop.sync_info = mybir.SyncInfo(on_wait=[w], on_update=[])
                        nc.register_instruction(nop, overwrite=True)
                        out.append(nop)
                    si.on_wait = waits[-1:]
                out.append(inst)
            if changed:
                bb.instructions = out


def _install_tilefix():
    import concourse.tile as tile_mod
    from concourse.vector_clock import ScopedClock

    if getattr(tile_mod.TileContext, "_rgcn_patched", False):
        return

    def patched_drain_and_barrier(self, tick_clock, wait_clock):
        nc = self.nc
        drain_inst = nc.sync.drain()
        wait_clock.add_sem_waits(
            drain_inst.ins, ScopedClock({None: tick_clock.global_clock}))
        nc.all_engine_barrier()
        assert self.sems is not None
        popped = nc._tile_sem_poison_stack.pop()
        assert popped is self._sem_poison
        nc.clear_and_free_semaphores(list(self.sems.allocated().values()))
        nc.all_engine_barrier()
        _split_multiwaits(nc)

    tile_mod.TileContext._drain_and_barrier = patched_drain_and_barrier
    tile_mod.TileContext._rgcn_patched = True


def _install_ntff_hook():
    import sys, types
    if 'antenv.axon_hooks' in sys.modules:
        return
    try:
        try:
            from trn_agent_boot.trn_boot import _ntff_profile_via_ctypes
        except ImportError:
            sys.path.insert(0, '/root/.axon_site')
            from trn_agent_boot.trn_boot import _ntff_profile_via_ctypes
        hook = _ntff_profile_via_ctypes('/opt/axon/libaxon_pjrt.so')
    except Exception:
        return
    mod = types.ModuleType('antenv.axon_hooks')
    mod.get_axon_ntff_profile_hook = lambda: hook
    mod.set_axon_ntff_profile_hook = lambda h: None
    sys.modules['antenv.axon_hooks'] = mod


# ---------------------------------------------------------------------------
# Host preprocessing: balanced node->tile packing + per-core index arrays
# ---------------------------------------------------------------------------
def _pack_core(d):
    """Greedy potential-based vector bin packing of one core's nodes with
    hard per-(relation, half) caps (128; last NOVER bins 256).
    d: [SHARD, 2R] per-node (relation, half) in-degree."""
    D = d.shape[1]
    caps = np.full((NT, D), 128, np.int32)
    caps[NT - NOVER:] = 256
    used = np.zeros((NT, D), np.int32)
    cnt = np.zeros(NT, np.int32)
    assign = np.empty(d.shape[0], np.int32)
    order = np.argsort(-(d.max(1).astype(np.int64) * 1000 + d.sum(1)),
                       kind='stable')
    for i in order:
        di = d[i]
        nu = used + di
        fits = (nu <= caps).all(1) & (cnt < 128)
        pot = (np.maximum(nu - 120, 0) ** 2).sum(1).astype(np.float64)
        pot += 0.02 * cnt.astype(np.float64) ** 1.5
        if fits.any():
            pot[~fits] = np.inf
            j = int(np.argmin(pot))
        else:
            sl = (caps - nu).min(1).astype(np.float64)
            sl[cnt >= 128] = -np.inf
            j = int(np.argmax(sl))
        assign[i] = j
        used[j] += di
        cnt[j] += 1
    return assign, used, cnt


def _build_partition(src, dst, et):
    """Pack every core's nodes into NT tiles; derive the shared per-tile
    (lo, hi) chunk schedule. Returns (node_tile, node_slot, sched) where
    sched is [NT, 2] chunk counts."""
    half = (src >= SPL1).astype(np.int64)
    deg = np.zeros((N, 2 * R), np.int32)
    np.add.at(deg, (dst, et * 2 + half), 1)

    node_tile = np.empty(N, np.int32)
    node_slot = np.empty(N, np.int32)
    all_needs = []
    assigns = []
    for c in range(NCORES):
        nodes = np.arange(c * SHARD, (c + 1) * SHARD)
        assign, used, cnt = _pack_core(deg[nodes])
        # need per (tile, half): chunks so every relation's half fits
        per = used.reshape(NT, R, 2)
        need = np.maximum(1, -(-per.max(1) // 128))      # [NT, 2]
        order = np.argsort(-need.sum(1), kind='stable')
        relabel = np.empty(NT, np.int32)
        relabel[order] = np.arange(NT)
        assigns.append(relabel[assign])
        all_needs.append(need[order])
    sched = np.max(all_needs, axis=0).astype(np.int64)   # [NT, 2]

    for c in range(NCORES):
        nodes = np.arange(c * SHARD, (c + 1) * SHARD)
        assign = assigns[c]
        order = np.argsort(assign, kind='stable')
        pos = np.empty(SHARD, np.int64)
        pos[order] = np.arange(SHARD)
        start = np.searchsorted(assign[order], np.arange(NT))
        node_tile[nodes] = assign
        node_slot[nodes] = (pos - start[assign]).astype(np.int32)
    return node_tile, node_slot, sched


def _host_prep(src, dst, et, node_tile, node_slot, sched):
    """Per-core gather/mask/scale arrays for the shared chunk schedule."""
    seg = et * N + dst
    cnt = np.bincount(seg, minlength=R * N).astype(np.float32)
    inv = np.where(cnt > 0, 1.0 / np.maximum(cnt, 1), 0.0).astype(np.float32)

    ncht = R * (sched[:, 0] + sched[:, 1])               # chunks per tile
    colbase = np.concatenate([[0], np.cumsum(ncht)])
    TOTCH = int(colbase[-1])
    TOTSLOT = TOTCH * 128

    core_of = dst // SHARD
    dtile = node_tile[dst]
    dslot = node_slot[dst].astype(np.float32)
    half = (src >= SPL1).astype(np.int64)

    pad_src = (src // SHARD) * PSH + node_tile[src] * 128 + node_slot[src]
    val1 = np.where(half == 0, src, src - SPL1)
    val2 = np.where(half == 0, pad_src, pad_src - SPL2)
    # src < SPL1 must coincide with pad_src < SPL2 (cores 0-3) for the shared
    # lo/hi split to be valid in both layers
    assert ((pad_src < SPL2) == (half == 0)).all()

    per_core = []
    for c in range(NCORES):
        eids = np.nonzero(core_of == c)[0]
        key = ((dtile[eids] * R + et[eids]) * 2 + half[eids]).astype(np.int64)
        order = np.argsort(key, kind='stable')
        eids = eids[order]
        key = key[order]
        starts = np.searchsorted(key, np.arange(NT * R * 2))
        ends = np.searchsorted(key, np.arange(NT * R * 2) + 1)

        slot_i1 = np.zeros(TOTSLOT, np.int64)
        slot_i2 = np.zeros(TOTSLOT, np.int64)
        slot_node = np.zeros(TOTSLOT, np.int64)
        slot_dl = np.full(TOTSLOT, -1.0, np.float32)
        slot_scale = np.zeros(TOTSLOT, np.float32)
        for t in range(NT):
            base = int(colbase[t]) * 128
            clo, chi = int(sched[t, 0]), int(sched[t, 1])
            for r in range(R):
                for h in (0, 1):
                    g = (t * R + r) * 2 + h
                    n = int(ends[g] - starts[g])
                    cap = (clo if h == 0 else chi) * 128
                    if n > cap:
                        raise RuntimeError(
                            f"overflow c{c} t{t} r{r} h{h}: {n} > {cap}")
                    e = eids[starts[g]:ends[g]]
                    if h == 0:
                        s0 = base + r * clo * 128
                    else:
                        s0 = base + (R * clo + r * chi) * 128
                    slot_i1[s0:s0 + n] = val1[e]
                    slot_i2[s0:s0 + n] = val2[e]
                    slot_node[s0:s0 + n] = src[e]
                    slot_dl[s0:s0 + n] = dslot[e]
                    slot_scale[s0:s0 + n] = inv[seg[e]]

        def idx_table(flat):
            # wrap each (tile, half) gather stripe: [16, n/16] replicated x8
            cols = np.zeros((128, TOTSLOT // 16), np.int16)
            for t in range(NT):
                s0 = int(colbase[t]) * 128
                nlo = int(sched[t, 0]) * R * 128
                nhi = int(sched[t, 1]) * R * 128
                for (o, n) in ((0, nlo), (nlo, nhi)):
                    blk = flat[s0 + o:s0 + o + n].astype(np.int16)
                    blk = blk.reshape(n // 16, 16).T
                    cols[:, (s0 + o) // 16:(s0 + o + n) // 16] = \
                        np.tile(blk, (8, 1))
            return np.ascontiguousarray(cols)

        # host-built masks with mean normalization folded in:
        # mask[p, col*128 + d] = (dl[slot]==d) * inv[seg(edge)]
        scale_slot = np.zeros(TOTSLOT, np.float32)
        valid = slot_dl >= 0
        vs = np.nonzero(valid)[0]
        # recover each valid slot's edge scale from its (tile, r) and dst
        # via dl and the per-(t,r) stored seg ids
        scale_slot[vs] = slot_scale[vs]
        marr = np.zeros((TOTCH, 128, 128), bf16)
        chunk_of = vs // 128
        part_of = vs % 128
        marr[chunk_of, part_of, slot_dl[vs].astype(np.int64)] = \
            scale_slot[vs].astype(bf16)
        mask_cols = np.ascontiguousarray(
            marr.transpose(1, 0, 2).reshape(128, TOTCH * 128))

        per_core.append(dict(
            snode=slot_node, idx2=idx_table(slot_i2),
            mask=mask_cols, totch=TOTCH))
    return per_core


def _pack_weights(W, nchunk):
    Rr, K, M = W.shape
    out = np.zeros((128, Rr * nchunk * M), bf16)
    for r in range(Rr):
        for c in range(nchunk):
            out[:, (r * nchunk + c) * M:(r * nchunk + c + 1) * M] = \
                W[r, c * 128:(c + 1) * 128, :].astype(bf16)
    return out


def _pack_single(Wm, nchunk):
    K, M = Wm.shape
    out = np.zeros((128, nchunk * M), bf16)
    for c in range(nchunk):
        out[:, c * M:(c + 1) * M] = Wm[c * 128:(c + 1) * 128, :].astype(bf16)
    return out


def _tiles_T(xf, c, width, node_tile, node_slot):
    """Transposed per-tile rows of core c: [128, (width/128)*PSH] bf16,
    block (c2, t) at columns (c2*NT + t)*128."""
    nch = width // 128
    nodes = np.arange(c * SHARD, (c + 1) * SHARD)
    rows = np.zeros((PSH, width), np.float32)
    rows[node_tile[nodes] * 128 + node_slot[nodes]] = xf[nodes]
    blk = rows.T.astype(bf16)
    out = np.zeros((128, nch * PSH), bf16)
    for c2 in range(nch):
        for t in range(NT):
            out[:, (c2 * NT + t) * 128:(c2 * NT + t + 1) * 128] = \
                blk[c2 * 128:(c2 + 1) * 128, t * 128:(t + 1) * 128]
    return out


# ---------------------------------------------------------------------------
# Device kernel builder
# ---------------------------------------------------------------------------
def _build_layer(layer, sched, totch):
    import concourse.bacc as bacc
    import concourse.mybir as mybir
    from concourse.tile import TileContext

    F = IN if layer == 1 else HID        # message width
    H = HID if layer == 1 else OUT       # output width
    FC = F // 128
    NSRC = N if layer == 1 else NCORES * PSH
    SPL = SPL1 if layer == 1 else SPL2
    TOTSLOT = totch * 128
    ncht = R * (sched[:, 0] + sched[:, 1])
    colbase = np.concatenate([[0], np.cumsum(ncht)])

    nc = bacc.Bacc("TRN2")
    # Layer 1's per-edge messages are materialized on the host as part of
    # input sharding (per the sharding hint) and streamed sequentially;
    # layer 2's messages depend on device-computed h, so they are gathered
    # on-device via dma_gather.
    if layer == 1:
        msgsd = nc.dram_tensor('msgsd', [128, totch * F], mybir.dt.bfloat16, kind='ExternalInput')
    else:
        xsrc = nc.dram_tensor('xsrc', [NSRC, F], mybir.dt.bfloat16, kind='ExternalInput')
        idx = nc.dram_tensor('idx', [128, TOTSLOT // 16], mybir.dt.int16, kind='ExternalInput')
    xT = nc.dram_tensor('xT', [128, FC * PSH], mybir.dt.bfloat16, kind='ExternalInput')
    Wsb = nc.dram_tensor('Wsb', [128, R * FC * H], mybir.dt.bfloat16, kind='ExternalInput')
    rootsb = nc.dram_tensor('rootsb', [128, FC * H], mybir.dt.bfloat16, kind='ExternalInput')
    brow = nc.dram_tensor('brow', [1, H], mybir.dt.bfloat16, kind='ExternalInput')
    maskd = nc.dram_tensor('maskd', [128, TOTSLOT], mybir.dt.bfloat16, kind='ExternalInput')
    out_dt = mybir.dt.bfloat16 if layer == 1 else mybir.dt.float32
    yout = nc.dram_tensor('yout', [PSH, H], out_dt, kind='ExternalOutput')

    with TileContext(nc) as tc:
        with tc.tile_pool(name='const', bufs=1) as cp, \
             tc.tile_pool(name='gather', bufs=5) as gp, \
             tc.tile_pool(name='masks', bufs=5) as mp, \
             tc.tile_pool(name='aggts', bufs=4) as atp, \
             tc.tile_pool(name='hout', bufs=3) as hp, \
             tc.tile_pool(name='pagg', bufs=4, space='PSUM') as pagg, \
             tc.tile_pool(name='pout', bufs=2, space='PSUM') as pout:

            xT_sb = cp.tile([128, FC * PSH], mybir.dt.bfloat16)
            nc.sync.dma_start(out=xT_sb[:], in_=xT[:])
            W_sb = cp.tile([128, R * FC * H], mybir.dt.bfloat16)
            nc.sync.dma_start(out=W_sb[:], in_=Wsb[:])
            root_sb = cp.tile([128, FC * H], mybir.dt.bfloat16)
            nc.sync.dma_start(out=root_sb[:], in_=rootsb[:])
            b_sb = cp.tile([1, H], mybir.dt.bfloat16)
            nc.sync.dma_start(out=b_sb[:], in_=brow[:])
            ones_sb = cp.tile([1, 128], mybir.dt.bfloat16)
            nc.vector.memset(ones_sb[:], 1.0)
            if layer != 1:
                idx_sb = cp.tile([128, TOTSLOT // 16], mybir.dt.int16)
                nc.sync.dma_start(out=idx_sb[:], in_=idx[:])

            for t in range(NT):
                clo, chi = int(sched[t, 0]), int(sched[t, 1])
                cb = int(colbase[t])
                nch = R * (clo + chi)
                nlo = R * clo * 128
                nhi = R * chi * 128
                msgs = gp.tile([128, nch, F], mybir.dt.bfloat16, tag='msgs')
                if layer == 1:
                    nc.sync.dma_start(
                        out=msgs[:, :, :],
                        in_=msgsd[:, cb * F:(cb + nch) * F])
                else:
                    nc.gpsimd.dma_gather(
                        out_ap=msgs[:, 0:R * clo, :], in_ap=xsrc[0:SPL, :],
                        idxs_ap=idx_sb[:, cb * 8:cb * 8 + nlo // 16],
                        num_idxs=nlo, num_idxs_reg=nlo, elem_size=F)
                    nc.gpsimd.dma_gather(
                        out_ap=msgs[:, R * clo:nch, :], in_ap=xsrc[SPL:NSRC, :],
                        idxs_ap=idx_sb[:, cb * 8 + nlo // 16:cb * 8 + (nlo + nhi) // 16],
                        num_idxs=nhi, num_idxs_reg=nhi, elem_size=F)
                msk = mp.tile([128, nch * 128], mybir.dt.bfloat16, tag='msk')
                nc.sync.dma_start(
                    out=msk[:], in_=maskd[:, cb * 128:(cb + nch) * 128])

                opsum = pout.tile([128, H], mybir.dt.float32)
                for r in range(R):
                    # aggregate directly in transposed (feature-major) form:
                    # aggT[:, c2*128+d] += msgs_chunk_c2.T @ mask_chunk
                    aggp = pagg.tile([128, F], mybir.dt.float32)
                    cols = [r * clo + ch for ch in range(clo)] + \
                           [R * clo + r * chi + ch for ch in range(chi)]
                    for c2 in range(FC):
                        for k, cc in enumerate(cols):
                            nc.tensor.matmul(
                                out=aggp[:, c2 * 128:(c2 + 1) * 128],
                                lhsT=msgs[:, cc, c2 * 128:(c2 + 1) * 128],
                                rhs=msk[:, cc * 128:(cc + 1) * 128],
                                start=(k == 0), stop=(k == len(cols) - 1))
                    aggT = atp.tile([128, F], mybir.dt.bfloat16, tag='aggT')
                    if r % 2 == 0:
                        nc.scalar.activation(
                            out=aggT[:], in_=aggp[:],
                            func=mybir.ActivationFunctionType.Copy)
                    else:
                        nc.vector.tensor_copy(out=aggT[:], in_=aggp[:])
                    for c2 in range(FC):
                        nc.tensor.matmul(
                            out=opsum[:],
                            lhsT=aggT[:, c2 * 128:(c2 + 1) * 128],
                            rhs=W_sb[:, (r * FC + c2) * H:(r * FC + c2 + 1) * H],
                            start=(r == 0 and c2 == 0), stop=False)
                for c2 in range(FC):
                    nc.tensor.matmul(
                        out=opsum[:],
                        lhsT=xT_sb[:, (c2 * NT + t) * 128:(c2 * NT + t + 1) * 128],
                        rhs=root_sb[:, c2 * H:(c2 + 1) * H],
                        start=False, stop=False)
                nc.tensor.matmul(
                    out=opsum[:], lhsT=ones_sb[:], rhs=b_sb[:],
                    start=False, stop=True)

                if layer == 1:
                    h_t = hp.tile([128, H], mybir.dt.bfloat16, tag='ht')
                    nc.scalar.activation(
                        out=h_t[:], in_=opsum[:],
                        func=mybir.ActivationFunctionType.Relu)
                    nc.sync.dma_start(
                        out=yout[t * 128:(t + 1) * 128, :], in_=h_t[:])
                else:
                    nrm2 = hp.tile([128, 1], mybir.dt.float32, tag='n2')
                    sq = hp.tile([128, OUT], mybir.dt.float32, tag='sq')
                    nc.scalar.activation(
                        out=sq[:], in_=opsum[:],
                        func=mybir.ActivationFunctionType.Square,
                        accum_out=nrm2[:])
                    srt = hp.tile([128, 1], mybir.dt.float32, tag='srt')
                    nc.scalar.activation(
                        out=srt[:], in_=nrm2[:],
                        func=mybir.ActivationFunctionType.Sqrt)
                    nc.vector.tensor_scalar_max(srt[:], srt[:], 1e-12)
                    rcp = hp.tile([128, 1], mybir.dt.float32, tag='rcp')
                    nc.vector.reciprocal(rcp[:], srt[:])
                    o_t = hp.tile([128, OUT], mybir.dt.float32, tag='ot')
                    nc.scalar.activation(
                        out=o_t[:], in_=opsum[:],
                        func=mybir.ActivationFunctionType.Copy,
                        scale=rcp[:])
                    nc.sync.dma_start(
                        out=yout[t * 128:(t + 1) * 128, :], in_=o_t[:])
    nc.compile()
    _split_multiwaits(nc)
    return nc


def _run(nc, in_maps, trace=False):
    global _last_traced
    from concourse import bass_utils
    res = bass_utils.run_bass_kernel_spmd(
        nc, in_maps, core_ids=list(range(NCORES)), trace=trace)
    if trace:
        _last_traced = res
    return res


# ---------------------------------------------------------------------------
# Entry point
# ---------------------------------------------------------------------------
def kernel(x, W1, root1, b1, W2, root2, b2, src, dst, edge_type,
           _trace=None):
    _install_tilefix()
    _install_ntff_hook()

    x = np.asarray(x, np.float32)
    src = np.asarray(src).astype(np.int64)
    dst = np.asarray(dst).astype(np.int64)
    et = np.asarray(edge_type).astype(np.int64)

    node_tile, node_slot, sched = _build_partition(src, dst, et)
    per_core = _host_prep(src, dst, et, node_tile, node_slot, sched)
    totch = per_core[0]['totch']

    x_bf = x.astype(bf16)
    W1p = _pack_weights(np.asarray(W1, np.float32), IN // 128)
    r1p = _pack_single(np.asarray(root1, np.float32), IN // 128)
    b1p = np.asarray(b1, np.float32)[None, :].astype(bf16)
    W2p = _pack_weights(np.asarray(W2, np.float32), HID // 128)
    r2p = _pack_single(np.asarray(root2, np.float32), HID // 128)
    b2p = np.asarray(b2, np.float32)[None, :].astype(bf16)

    # ---- layer 1 ----
    nc1 = _build_layer(1, sched, totch)
    in_maps1 = []
    for c in range(NCORES):
        pc = per_core[c]
        msgs1 = np.ascontiguousarray(
            x_bf[pc['snode']].reshape(totch, 128, IN)
            .transpose(1, 0, 2).reshape(128, totch * IN))
        in_maps1.append(dict(
            msgsd=msgs1, xT=_tiles_T(x, c, IN, node_tile, node_slot),
            Wsb=W1p, rootsb=r1p, brow=b1p, maskd=pc['mask']))
    res1 = _run(nc1, in_maps1, trace=(_trace == 'l1_0'))
    _pending_trace['l1'] = res1.exec_time_ns
    h_full = np.concatenate([res1.results[c]['yout'] for c in range(NCORES)])

    # ---- layer 2 ----
    h_f32 = h_full.astype(np.float32)
    nc2 = _build_layer(2, sched, totch)
    in_maps2 = []
    for c in range(NCORES):
        pc = per_core[c]
        blk = h_f32[c * PSH:(c + 1) * PSH].T.astype(bf16)   # [HID, PSH]
        hT = np.zeros((128, (HID // 128) * PSH), bf16)
        for c2 in range(HID // 128):
            for t in range(NT):
                hT[:, (c2 * NT + t) * 128:(c2 * NT + t + 1) * 128] = \
                    blk[c2 * 128:(c2 + 1) * 128, t * 128:(t + 1) * 128]
        in_maps2.append(dict(
            xsrc=h_full, xT=hT, Wsb=W2p, rootsb=r2p, brow=b2p,
            idx=pc['idx2'], maskd=pc['mask']))
    res2 = _run(nc2, in_maps2, trace=(_trace == 'l2_0'))
    _pending_trace['l2'] = res2.exec_time_ns

    out = np.empty((N, OUT), np.float32)
    for c in range(NCORES):
        nodes = np.arange(c * SHARD, (c + 1) * SHARD)
        rows = node_tile[nodes] * 128 + node_slot[nodes]
        out[nodes] = res2.results[c]['yout'][rows].astype(np.float32)
    return out
